# revision 1
# baseline (speedup 1.0000x reference)
"""Trainium2 Bass kernel for nn_MoETransformerDecoderFFN_84026740178981.

Expert-parallel across 8 NeuronCores: core e computes expert e over the full
batch; host sums the 8 per-core weighted outputs.

v2: fused single-pass pipeline (no phase teardown, no ln1 DRAM bounce),
bf16 weights + activations everywhere except the gating chain (fp32r —
top-2 selection is a discrete cliff), token-major top-2 gating, paired
2-bank PSUM exp tiles, LN rows fp32 with bf16 broadcast rows.

Self-contained: builds + compiles the Bass program on first call (cached at
module level), runs via PJRT on jax.devices()[:8].
"""
import sys

if '/opt/trn_rl_repo' not in sys.path:
    sys.path.insert(0, '/opt/trn_rl_repo')

import numpy as np

from contextlib import ExitStack

import concourse.bass as bass
import concourse.mybir as mybir
import concourse.tile as tile
from concourse import bacc
from concourse.masks import make_identity

F32 = mybir.dt.float32
F32R = mybir.dt.float32r
BF16 = mybir.dt.bfloat16
F8 = mybir.dt.float8e4
AF = mybir.ActivationFunctionType
ALU = mybir.AluOpType


def build(B=4, S=1024, D=512, F=2048, E=8, NH=8, n_cores=8, NT=512,
          gelu_func=None, loop=1, pt_f32=False, spine_f32=False,
          h1_f32=False, y_f32=False, rows_f32=False, shrink=False):
    ROW_DT = F32 if rows_f32 else BF16
    XB = 1 if shrink else 2
    del XB
    PT_DT = F32R if pt_f32 else BF16
    SP_DT = F32R if spine_f32 else BF16
    H1_DT = F32R if h1_f32 else BF16
    Y_DT = F32 if y_f32 else BF16
    W2_DT = F32R if h1_f32 else BF16
    HD = D // NH            # head dim (64)
    HPT = 128 // HD         # heads per 128-partition tile (2)
    T = B * S
    KC = D // 128           # feature chunks of D (4)
    FC = F // 128           # feature chunks of F (16)
    NT = min(NT, S)
    SB = S // NT            # token blocks per batch
    NKT = S // 128          # k-token tiles per batch (8)
    JT = D // 128           # output-feature tiles of D (4)
    CHK = S // 128          # 128-token chunks per batch (8)
    eps = 1e-5
    if gelu_func is None:
        gelu_func = AF.Gelu

    nc = bacc.Bacc("TRN2", target_bir_lowering=False, debug=False,
                   num_devices=n_cores)

    # ---- DRAM I/O ----
    d_xr = nc.dram_tensor("xr", [D, T], F32R, kind="ExternalInput")
    d_gA = nc.dram_tensor("gA", [D, E], F32, kind="ExternalInput")    # Wk^T eq^T D^-.5
    d_gc = nc.dram_tensor("gc", [E], F32, kind="ExternalInput")        # eq bk D^-.5
    d_sel = nc.dram_tensor("sel", [E], F32, kind="ExternalInput")      # one-hot(expert)
    d_wqT = nc.dram_tensor("wqT", [D, D], F32R, kind="ExternalInput")  # pre-scaled HD^-.5
    d_wkT = nc.dram_tensor("wkT", [D, D], F32R, kind="ExternalInput")
    d_wvT = nc.dram_tensor("wvT", [D, D], F32R, kind="ExternalInput")
    d_bq = nc.dram_tensor("bq", [D], F32, kind="ExternalInput")        # pre-scaled
    d_bk = nc.dram_tensor("bk", [D], F32, kind="ExternalInput")
    d_bv = nc.dram_tensor("bv", [D], F32, kind="ExternalInput")
    d_woT = nc.dram_tensor("woT", [D, D], BF16, kind="ExternalInput")
    d_bo = nc.dram_tensor("bo", [D], F32, kind="ExternalInput")
    d_g1 = nc.dram_tensor("g1", [D], F32, kind="ExternalInput")
    d_be1 = nc.dram_tensor("be1", [D], F32, kind="ExternalInput")
    d_w1T = nc.dram_tensor("w1T", [D, F], BF16, kind="ExternalInput")
    d_bf1 = nc.dram_tensor("bf1", [F], F32, kind="ExternalInput")
    d_w2T = nc.dram_tensor("w2T", [F, D], W2_DT, kind="ExternalInput")
    d_bf2 = nc.dram_tensor("bf2", [D], F32, kind="ExternalInput")
    d_g2 = nc.dram_tensor("g2", [D], F32, kind="ExternalInput")
    d_be2 = nc.dram_tensor("be2", [D], F32, kind="ExternalInput")
    d_yT = nc.dram_tensor("yT", [D, T], Y_DT, kind="ExternalOutput")

    def pcol(dram_1d, c):
        """[len] DRAM vector -> [128, c] partition-major view."""
        return dram_1d.rearrange("(c p) -> p c", p=128)

    with tile.TileContext(nc) as tc, ExitStack() as top:
        const = top.enter_context(tc.tile_pool(name="const", bufs=1))

        # ---- whole-kernel constants / weights ----
        ident = const.tile([128, 128], F32)
        make_identity(nc, ident)
        ones_bf = const.tile([128, 1], BF16)
        nc.vector.memset(ones_bf, 1.0)
        if spine_f32:
            ones_f32t = const.tile([128, 1], F32)
            nc.vector.memset(ones_f32t, 1.0)
            ones_ln = const.tile([128, 1], F32R)
            nc.vector.tensor_copy(ones_ln, ones_f32t)
        else:
            ones_ln = ones_bf
        eps_sb = const.tile([1, 1], F32)
        nc.vector.memset(eps_sb, eps)
        sel_bc = const.tile([128, E], F32)
        nc.sync.dma_start(
            out=sel_bc.rearrange("p (o e) -> p o e", o=1),
            in_=d_sel.rearrange("(o e) -> o e", o=1).partition_broadcast(128))
        gc_bc = const.tile([128, E], F32)
        nc.sync.dma_start(
            out=gc_bc.rearrange("p (o e) -> p o e", o=1),
            in_=d_gc.rearrange("(o e) -> o e", o=1).partition_broadcast(128))
        bv_bc = const.tile([128, D], F32)
        nc.sync.dma_start(
            out=bv_bc.rearrange("p (o d) -> p o d", o=1),
            in_=d_bv.rearrange("(o d) -> o d", o=1).partition_broadcast(128))

        gA_sb = const.tile([128, KC, E], F32)
        nc.sync.dma_start(out=gA_sb,
                          in_=d_gA.rearrange("(c p) e -> p c e", p=128))
        wv_sb = const.tile([128, KC, D], F32R)
        nc.sync.dma_start(out=wv_sb,
                          in_=d_wvT.rearrange("(c p) j -> p c j", p=128))
        wq_sb = const.tile([128, KC, D], F32R)
        nc.sync.dma_start(out=wq_sb,
                          in_=d_wqT.rearrange("(c p) j -> p c j", p=128))
        wk_sb = const.tile([128, KC, D], F32R)
        nc.sync.dma_start(out=wk_sb,
                          in_=d_wkT.rearrange("(c p) j -> p c j", p=128))
        wo_sb = const.tile([128, KC, D], BF16)
        nc.sync.dma_start(out=wo_sb,
                          in_=d_woT.rearrange("(c p) j -> p c j", p=128))

        bq_sb = const.tile([128, KC], F32)
        nc.sync.dma_start(out=bq_sb, in_=pcol(d_bq, KC))
        bk_sb = const.tile([128, KC], F32)
        nc.sync.dma_start(out=bk_sb, in_=pcol(d_bk, KC))
        bo_sb = const.tile([128, KC], F32)
        nc.sync.dma_start(out=bo_sb, in_=pcol(d_bo, KC))
        g1_sb = const.tile([128, KC], F32)
        nc.sync.dma_start(out=g1_sb, in_=pcol(d_g1, KC))
        be1_sb = const.tile([128, KC], F32)
        nc.sync.dma_start(out=be1_sb, in_=pcol(d_be1, KC))
        g2_sb = const.tile([128, KC], F32)
        nc.sync.dma_start(out=g2_sb, in_=pcol(d_g2, KC))
        be2_sb = const.tile([128, KC], F32)
        nc.sync.dma_start(out=be2_sb, in_=pcol(d_be2, KC))
        bf1_sb = const.tile([128, FC], F32)
        nc.sync.dma_start(out=bf1_sb, in_=pcol(d_bf1, FC))
        bf2_sb = const.tile([128, KC], F32)
        nc.sync.dma_start(out=bf2_sb, in_=pcol(d_bf2, KC))
        w1_sb = const.tile([128, KC, F], BF16)
        nc.sync.dma_start(out=w1_sb,
                          in_=d_w1T.rearrange("(c p) j -> p c j", p=128))
        w2_sb = const.tile([128, FC, D], W2_DT)
        nc.sync.dma_start(out=w2_sb,
                          in_=d_w2T.rearrange("(c p) j -> p c j", p=128))

        with ExitStack() as mp:
            # SBUF pools
            poolx = mp.enter_context(tc.tile_pool(name="px", bufs=2 if not shrink else 1))
            poolg = mp.enter_context(tc.tile_pool(name="pg", bufs=1))
            poolv = mp.enter_context(tc.tile_pool(name="pv", bufs=1))
            poolq = mp.enter_context(tc.tile_pool(name="pq", bufs=2))
            poolp = mp.enter_context(tc.tile_pool(name="pp", bufs=2))
            poolc = mp.enter_context(tc.tile_pool(name="pc", bufs=2 if not shrink else 1))
            pooll = mp.enter_context(tc.tile_pool(name="pl", bufs=2 if not shrink else 1))
            poolr = mp.enter_context(tc.tile_pool(name="pr", bufs=2 if not shrink else 1))
            pools1 = mp.enter_context(tc.tile_pool(name="ps1", bufs=1))
            poolh = mp.enter_context(tc.tile_pool(name="ph", bufs=1))
            pooly = mp.enter_context(tc.tile_pool(name="py", bufs=1))
            poolw = mp.enter_context(tc.tile_pool(name="pw", bufs=2))
            # PSUM pools: mm(2) + sc(2x2 banks) + ctx(2) = 8 banks
            psmm = mp.enter_context(tc.tile_pool(name="psmm", bufs=2, space="PSUM"))
            pssc = mp.enter_context(tc.tile_pool(name="pssc", bufs=2, space="PSUM"))
            psctx = mp.enter_context(tc.tile_pool(name="psctx", bufs=2, space="PSUM"))

            def ln_rows(pst2, tag):
                """PSUM [33,NT] (sum at row 0, sq-sum at row 32) ->
                (rstd_bf, nmr_bf) [1,NT] bf16 rows.
                Quadrant rows: 0 mean, 32 var, 64 msq/rstd, 96 std.
                Two-SBUF-input ops must share base partition, so the
                cross-quadrant combines read pst2 (PSUM operand exempt)."""
                r = poolw.tile([97, NT], F32, tag="lnrows")
                rb_r = poolw.tile([1, NT], ROW_DT, tag="lnrstd")
                rb_n = poolw.tile([1, NT], ROW_DT, tag="lnnmr")
                nc.vector.tensor_scalar_mul(r[0:1, :], pst2[0:1, :], 1.0 / D)
                nc.vector.tensor_tensor(r[64:65, :], r[0:1, :], r[0:1, :],
                                        ALU.mult)
                nc.vector.scalar_tensor_tensor(
                    r[32:33, :], pst2[32:33, :], 1.0 / D, r[64:65, :],
                    ALU.mult, ALU.subtract)
                nc.scalar.activation(r[96:97, :], r[32:33, :], AF.Sqrt,
                                     bias=eps_sb)
                nc.vector.reciprocal(r[64:65, :], r[96:97, :])
                nc.vector.tensor_copy(rb_r, r[64:65, :])
                nc.vector.scalar_tensor_tensor(rb_n, pst2[0:1, :], -1.0 / D,
                                               r[64:65, :], ALU.mult, ALU.mult)
                return rb_r, rb_n

            for _loop in range(loop):
                def xr_dma(b):
                    """Issue the batch x load early."""
                    tok0 = b * S
                    xr = poolx.tile([128, KC, S], F32R, tag="xr")
                    nc.sync.dma_start(
                        out=xr,
                        in_=d_xr.rearrange("(c p) t -> p c t", p=128)[
                            :, :, tok0:tok0 + S])
                    return xr

                def stage1(b, xr):
                    """Gating top-2 chain -> wown. Scores use an exact-f32
                    matmul (fp32r HW matmul precision flips top-2 picks)."""
                    s_tok = poolg.tile([128, CHK, E], F32, tag="stok")
                    for c4 in range(CHK):
                        pst = psmm.tile([128, E], F32, tag="mm")
                        for kc in range(KC):
                            nc.tensor.matmul(
                                pst,
                                xr[:, kc, bass.ts(c4, 128)].bitcast(F32),
                                gA_sb[:, kc, :],
                                start=(kc == 0), stop=(kc == KC - 1))
                        nc.vector.tensor_add(s_tok[:, c4, :], pst, gc_bc)

                    def ebc(t):  # [128, CHK] -> bcast over expert axis
                        return t.rearrange("p c -> p c ()").broadcast_to(
                            [128, CHK, E])

                    m1 = poolg.tile([128, CHK], F32, tag="m1")
                    nc.vector.reduce_max(m1, s_tok, axis=mybir.AxisListType.X)
                    geb = poolg.tile([128, CHK, E], F32, tag="geb")
                    nc.vector.tensor_tensor(geb, s_tok, ebc(m1), ALU.is_ge)
                    km = poolg.tile([128, CHK, E], F32, tag="km")
                    nc.vector.scalar_tensor_tensor(km, geb, -1e9, s_tok,
                                                   ALU.mult, ALU.add)
                    m2 = poolg.tile([128, CHK], F32, tag="m2")
                    nc.vector.reduce_max(m2, km, axis=mybir.AxisListType.X)
                    mask = poolg.tile([128, CHK, E], F32, tag="mask")
                    nc.vector.tensor_tensor(mask, s_tok, ebc(m2), ALU.is_ge)
                    pex = poolg.tile([128, CHK, E], F32, tag="pex")
                    nc.scalar.activation(pex, s_tok, AF.Exp)
                    gm = poolg.tile([128, CHK, E], F32, tag="gm")
                    nc.vector.tensor_tensor(gm, pex, mask, ALU.mult)
                    gsel = poolg.tile([128, CHK, E], F32, tag="gsel")
                    nc.vector.tensor_tensor(
                        gsel, gm,
                        sel_bc.rearrange("p e -> p () e").broadcast_to(
                            [128, CHK, E]), ALU.mult)
                    wnum = poolg.tile([128, CHK], F32, tag="wnum")
                    nc.vector.reduce_sum(wnum, gsel, axis=mybir.AxisListType.X)
                    dn = poolg.tile([128, CHK], F32, tag="dn")
                    nc.vector.reduce_sum(dn, gm, axis=mybir.AxisListType.X)
                    rc = poolg.tile([128, CHK], F32, tag="rc")
                    nc.vector.reciprocal(rc, dn)
                    wown = poolg.tile([128, CHK], F32, tag="wown")
                    nc.vector.tensor_tensor(wown, wnum, rc, ALU.mult)
                    return wown

                def stage2(b, xr):
                    """v projection -> v_t."""
                    v_t = poolv.tile([128, NKT, NH, HD + 1], PT_DT, tag="vt")
                    nc.vector.tensor_copy(
                        v_t[:, :, :, HD:HD + 1],
                        (ones_bf if not pt_f32 else
                         ones_ln).broadcast_to([128, NKT, NH, 1]))
                    for tt in range(NKT):
                        ps = psmm.tile([128, D], F32, tag="mm")
                        for kc in range(KC):
                            nc.tensor.matmul(
                                ps, xr[:, kc, bass.ts(tt, 128)],
                                wv_sb[:, kc, :],
                                start=(kc == 0), stop=(kc == KC - 1))
                        nc.vector.tensor_add(
                            v_t[:, tt, :, 0:HD],
                            ps.rearrange("p (h d) -> p h d", h=NH),
                            bv_bc.rearrange("p (h d) -> p h d", h=NH))
                    return v_t

                def stage3(b, xr, v_t):
                    """q/k projection + attention -> ctxT."""
                    ctxT = poolc.tile([128, KC, S], BF16, tag="ctxT")
                    for jt in range(JT):
                        qp = poolq.tile([128, S], BF16, tag="qp")
                        kp = poolq.tile([128, S], BF16, tag="kp")
                        for qb in range(SB):
                            ts = bass.ts(qb, NT)
                            psq = psmm.tile([128, NT], F32, tag="mm")
                            for kc in range(KC):
                                nc.tensor.matmul(
                                    psq, wq_sb[:, kc, bass.ts(jt, 128)],
                                    xr[:, kc, ts],
                                    start=(kc == 0), stop=(kc == KC - 1))
                            nc.vector.tensor_scalar_add(qp[:, ts], psq,
                                                        bq_sb[:, jt:jt + 1])
                            psk = psmm.tile([128, NT], F32, tag="mm")
                            for kc in range(KC):
                                nc.tensor.matmul(
                                    psk, wk_sb[:, kc, bass.ts(jt, 128)],
                                    xr[:, kc, ts],
                                    start=(kc == 0), stop=(kc == KC - 1))
                            nc.vector.tensor_scalar_add(kp[:, ts], psk,
                                                        bk_sb[:, jt:jt + 1])
                        for hh in range(HPT):
                            h = jt * HPT + hh
                            hp = bass.ds(hh * HD, HD)
                            for qb in range(SB):
                                ts = bass.ts(qb, NT)
                                psc = psctx.tile([HD + 1, NT], F32, tag="ctx")
                                for k2 in range(NKT // 2):
                                    pss = pssc.tile([128, 2, NT], F32,
                                                    tag="sc")
                                    nc.tensor.matmul(
                                        pss[:, 0, :],
                                        kp[hp, bass.ts(2 * k2, 128)],
                                        qp[hp, ts], start=True, stop=True)
                                    nc.tensor.matmul(
                                        pss[:, 1, :],
                                        kp[hp, bass.ts(2 * k2 + 1, 128)],
                                        qp[hp, ts], start=True, stop=True)
                                    pt = poolp.tile([128, 2, NT], PT_DT,
                                                    tag="pt")
                                    nc.scalar.activation(pt, pss, AF.Exp)
                                    nc.tensor.matmul(
                                        psc, v_t[:, 2 * k2, h, :],
                                        pt[:, 0, :],
                                        start=(k2 == 0), stop=False)
                                    nc.tensor.matmul(
                                        psc, v_t[:, 2 * k2 + 1, h, :],
                                        pt[:, 1, :],
                                        start=False,
                                        stop=(k2 == NKT // 2 - 1))
                                rr = poolp.tile([1, NT], ROW_DT, tag="rr")
                                with nc.allow_low_precision(
                                        reason="bf16 attn denominator"):
                                    nc.vector.reciprocal(
                                        rr, psc[HD:HD + 1, :])
                                rb = poolp.tile([HD, NT], ROW_DT, tag="rb")
                                nc.gpsimd.partition_broadcast(rb, rr,
                                                              channels=HD)
                                nc.vector.tensor_tensor(
                                    ctxT[bass.ds(hh * HD, HD), jt, ts],
                                    psc[0:HD, :], rb, ALU.mult)
                    return ctxT

                def stage4(b, xr, ctxT, wown):
                    """Wo + residual + LN1 + gate row -> (ln1, g_row)."""
                    pswt = psmm.tile([CHK, 128], F32, tag="mm")
                    nc.tensor.transpose(pswt, wown, ident)
                    g_sb = pools1.tile([CHK, 128], ROW_DT, tag="gsb")
                    nc.vector.tensor_copy(g_sb, pswt)
                    g_row = pools1.tile([1, S], ROW_DT, tag="grow")
                    nc.sync.dma_start(
                        out=g_row.rearrange("o (c j) -> o c j", c=CHK),
                        in_=g_sb.rearrange("c j -> c () j"))
                    ln1 = pooll.tile([128, KC, S], BF16, tag="ln1")
                    for qb in range(SB):
                        ts = bass.ts(qb, NT)
                        r1 = poolr.tile([128, KC, NT], SP_DT, tag="r1")
                        sq = pools1.tile([128, KC, NT], SP_DT, tag="sq")
                        for jt in range(JT):
                            ps = psmm.tile([128, NT], F32, tag="mm")
                            for kc in range(KC):
                                nc.tensor.matmul(
                                    ps, wo_sb[:, kc, bass.ts(jt, 128)],
                                    ctxT[:, kc, ts],
                                    start=(kc == 0), stop=(kc == KC - 1))
                            nc.vector.scalar_tensor_tensor(
                                r1[:, jt, :], ps, bo_sb[:, jt:jt + 1],
                                xr[:, jt, ts], ALU.add, ALU.add)
                            nc.vector.tensor_tensor(
                                sq[:, jt, :], r1[:, jt, :], r1[:, jt, :],
                                ALU.mult)
                        pst2 = psctx.tile([33, NT], F32, tag="ctx")
                        for kc in range(KC):
                            nc.tensor.matmul(pst2[0:1, :], ones_ln,
                                             r1[:, kc, :],
                                             start=(kc == 0),
                                             stop=(kc == KC - 1))
                        for kc in range(KC):
                            nc.tensor.matmul(pst2[32:33, :], ones_ln,
                                             sq[:, kc, :],
                                             start=(kc == 0),
                                             stop=(kc == KC - 1))
                        rstd_bf, nmr_bf = ln_rows(pst2, "1")
                        rsb = poolw.tile([128, NT], ROW_DT, tag="rsb")
                        nc.gpsimd.partition_broadcast(rsb, rstd_bf,
                                                      channels=128)
                        nsb = poolw.tile([128, NT], ROW_DT, tag="nsb")
                        nc.gpsimd.partition_broadcast(nsb, nmr_bf,
                                                      channels=128)
                        for kc in range(KC):
                            tmp = pools1.tile([128, NT], BF16, tag="lntmp")
                            nc.vector.tensor_tensor(tmp, r1[:, kc, :], rsb,
                                                    ALU.mult)
                            nc.vector.tensor_tensor(tmp, tmp, nsb, ALU.add)
                            nc.vector.tensor_scalar(
                                ln1[:, kc, ts], tmp, g1_sb[:, kc:kc + 1],
                                be1_sb[:, kc:kc + 1], ALU.mult, ALU.add)
                    return ln1, g_row

                def stage5(b, ln1, g_row):
                    """FFN + LN2 + gate multiply + output DMA."""
                    tok0 = b * S
                    for qb in range(SB):
                        ts = bass.ts(qb, NT)
                        h1 = poolh.tile([128, FC, NT], H1_DT, tag="h1")
                        for ft in range(FC):
                            ps = psmm.tile([128, NT], F32, tag="mm")
                            for kc in range(KC):
                                nc.tensor.matmul(
                                    ps, w1_sb[:, kc, bass.ts(ft, 128)],
                                    ln1[:, kc, ts],
                                    start=(kc == 0), stop=(kc == KC - 1))
                            nc.scalar.activation(h1[:, ft, :], ps, gelu_func,
                                                 bias=bf1_sb[:, ft:ft + 1])
                        r2 = poolr.tile([128, KC, NT], SP_DT, tag="r1")
                        sq2 = pools1.tile([128, KC, NT], SP_DT, tag="sq")
                        for jt in range(JT):
                            ps = psmm.tile([128, NT], F32, tag="mm")
                            for fc in range(FC):
                                nc.tensor.matmul(
                                    ps, w2_sb[:, fc, bass.ts(jt, 128)],
                                    h1[:, fc, :],
                                    start=(fc == 0), stop=(fc == FC - 1))
                            g2t = poolr.tile([128, NT], BF16, tag="g2t")
                            nc.scalar.activation(g2t, ps, gelu_func,
                                                 bias=bf2_sb[:, jt:jt + 1])
                            nc.vector.tensor_tensor(
                                r2[:, jt, :], ln1[:, jt, ts], g2t, ALU.add)
                            nc.vector.tensor_tensor(
                                sq2[:, jt, :], r2[:, jt, :], r2[:, jt, :],
                                ALU.mult)
                        pst2 = psctx.tile([33, NT], F32, tag="ctx")
                        for kc in range(KC):
                            nc.tensor.matmul(pst2[0:1, :], ones_ln,
                                             r2[:, kc, :],
                                             start=(kc == 0),
                                             stop=(kc == KC - 1))
                        for kc in range(KC):
                            nc.tensor.matmul(pst2[32:33, :], ones_ln,
                                             sq2[:, kc, :],
                                             start=(kc == 0),
                                             stop=(kc == KC - 1))
                        rstd_bf, nmr_bf = ln_rows(pst2, "2")
                        rsb2 = poolw.tile([128, NT], ROW_DT, tag="rsb")
                        nc.gpsimd.partition_broadcast(rsb2, rstd_bf,
                                                      channels=128)
                        nsb2 = poolw.tile([128, NT], ROW_DT, tag="nsb")
                        nc.gpsimd.partition_broadcast(nsb2, nmr_bf,
                                                      channels=128)
                        gb = poolw.tile([128, NT], ROW_DT, tag="gb")
                        nc.gpsimd.partition_broadcast(
                            gb, g_row[0:1, bass.ts(qb, NT)], channels=128)
                        yt = pooly.tile([128, KC, NT], Y_DT, tag="yt")
                        for kc in range(KC):
                            tmp = pools1.tile([128, NT], BF16, tag="lntmp")
                            nc.vector.tensor_tensor(tmp, r2[:, kc, :], rsb2,
                                                    ALU.mult)
                            nc.vector.tensor_tensor(tmp, tmp, nsb2, ALU.add)
                            nc.vector.tensor_scalar(
                                tmp, tmp, g2_sb[:, kc:kc + 1],
                                be2_sb[:, kc:kc + 1], ALU.mult, ALU.add)
                            nc.vector.tensor_tensor(yt[:, kc, :], tmp, gb,
                                                    ALU.mult)
                        nc.sync.dma_start(
                            out=d_yT.rearrange("(c p) t -> p c t", p=128)[
                                :, :, tok0 + qb * NT:tok0 + (qb + 1) * NT],
                            in_=yt)

                # software pipeline: next batch's gating + v-proj are
                # emitted before the previous batch's FFN so PE always has
                # independent work across the LN1 row-latency.
                pending = None
                for b in range(B):
                    xr = xr_dma(b)
                    if pending is not None:
                        stage5(*pending)
                    wown = stage1(b, xr)
                    v_t = stage2(b, xr)
                    ctxT = stage3(b, xr, v_t)
                    ln1, g_row = stage4(b, xr, ctxT, wown)
                    pending = (b, ln1, g_row)
                stage5(*pending)
    nc.compile()
    return nc


def make_in_map(inputs, e, B=4, S=1024, D=512, F=2048, E=8, NH=8):
    """Host-side input marshalling for core `e` (expert `e`)."""
    HD = D // NH
    f32 = np.float32
    bf16 = np.dtype('bfloat16') if hasattr(np, 'bfloat16') else None
    import ml_dtypes
    bf16 = ml_dtypes.bfloat16
    xT = np.ascontiguousarray(np.asarray(inputs["x"], f32).reshape(-1, D).T)
    Wqkv = np.asarray(inputs["Wqkv"][e], f32)
    bqkv = np.asarray(inputs["bqkv"][e], f32)
    WqkvT = Wqkv.T
    scale = f32(1.0 / np.sqrt(HD))
    gwk = np.asarray(inputs["gate_Wk"], np.float64)
    gbk = np.asarray(inputs["gate_bk"], np.float64)
    eq = np.asarray(inputs["expert_queries"], np.float64)
    gA = (gwk.T @ eq.T) * (D ** -0.5)            # [D, E]
    gc = (eq @ gbk) * (D ** -0.5)                # [E]
    return {
        "xr": xT,
        "gA": np.ascontiguousarray(gA.astype(f32)),
        "gc": np.ascontiguousarray(gc.astype(f32)),
        "sel": np.eye(E, dtype=f32)[e],
        "wqT": np.ascontiguousarray(WqkvT[:, :D] * scale),
        "wkT": np.ascontiguousarray(WqkvT[:, D:2 * D]),
        "wvT": np.ascontiguousarray(WqkvT[:, 2 * D:]),
        "bq": np.ascontiguousarray(bqkv[:D] * scale),
        "bk": np.ascontiguousarray(bqkv[D:2 * D]),
        "bv": np.ascontiguousarray(bqkv[2 * D:]),
        "woT": np.ascontiguousarray(
            np.asarray(inputs["Wo"][e], f32).T).astype(bf16),
        "bo": np.asarray(inputs["bo"][e], f32),
        "g1": np.asarray(inputs["g1"][e], f32),
        "be1": np.asarray(inputs["be1"][e], f32),
        "w1T": np.ascontiguousarray(
            np.asarray(inputs["W1"][e], f32).T).astype(bf16),
        "bf1": np.asarray(inputs["bf1"][e], f32),
        "w2T": np.ascontiguousarray(
            np.asarray(inputs["W2"][e], f32).T).astype(bf16),
        "bf2": np.asarray(inputs["bf2"][e], f32),
        "g2": np.asarray(inputs["g2"][e], f32),
        "be2": np.asarray(inputs["be2"][e], f32),
    }


class SpmdRunner:
    def __init__(self, nc, n_cores=8):
        import jax
        from jax.sharding import Mesh, PartitionSpec, NamedSharding
        from jax.experimental.shard_map import shard_map
        import concourse.mybir as mybir
        from concourse import bass2jax

        bass2jax.install_neuronx_cc_hook()
        self.jax = jax
        self.nc = nc
        self.n_cores = n_cores

        partition_name = (nc.partition_id_tensor.name
                          if nc.partition_id_tensor else None)
        in_names, out_names, out_avals, zero_outs = [], [], [], []
        for alloc in nc.m.functions[0].allocations:
            if not isinstance(alloc, mybir.MemoryLocationSet):
                continue
            name = alloc.memorylocations[0].name
            if alloc.kind == "ExternalInput":
                if name != partition_name:
                    in_names.append(name)
            elif alloc.kind == "ExternalOutput":
                shape = tuple(alloc.tensor_shape)
                dtype = mybir.dt.np(alloc.dtype)
                out_names.append(name)
                out_avals.append(jax.core.ShapedArray(shape, dtype))
                zero_outs.append(np.zeros(shape, dtype))
        self.in_names, self.out_names = in_names, out_names
        self.out_avals, self.zero_outs = out_avals, zero_outs
        n_params, n_outs = len(in_names), len(out_names)
        all_in_names = list(in_names) + list(out_names)
        if partition_name is not None:
            all_in_names.append(partition_name)

        def _body(*args):
            operands = list(args)
            if partition_name is not None:
                operands.append(bass2jax.partition_id_tensor())
            outs = bass2jax._bass_exec_p.bind(
                *operands,
                out_avals=tuple(out_avals),
                in_names=tuple(all_in_names),
                out_names=tuple(out_names),
                lowering_input_output_aliases=(),
                sim_require_finite=True,
                sim_require_nnan=True,
                nc=nc,
            )
            return tuple(outs)

        devices = jax.devices()[:n_cores]
        assert len(devices) == n_cores
        self.mesh = Mesh(np.asarray(devices), ("core",))
        specs = (PartitionSpec("core"),) * (n_params + n_outs)
        out_specs = (PartitionSpec("core"),) * n_outs
        self.sharding = NamedSharding(self.mesh, PartitionSpec("core"))
        self.fn = jax.jit(
            shard_map(_body, mesh=self.mesh, in_specs=specs,
                      out_specs=out_specs, check_rep=False),
            keep_unused=True)
        self._dev_args = None

    def set_inputs(self, in_maps):
        """in_maps: list of dicts (one per core). Transfers to device once."""
        jax = self.jax
        per_core = [[np.asarray(m[name]) for name in self.in_names]
                    for m in in_maps]
        concat = [np.concatenate([per_core[c][i] for c in range(self.n_cores)],
                                 axis=0)
                  for i in range(len(self.in_names))]
        concat += [np.zeros((self.n_cores * z.shape[0], *z.shape[1:]), z.dtype)
                   for z in self.zero_outs]
        self._dev_args = [jax.device_put(a, self.sharding) for a in concat]
        return self

    def run(self):
        outs = self.fn(*self._dev_args)
        self.jax.block_until_ready(outs)
        return outs

    def results(self, outs):
        out = []
        for c in range(self.n_cores):
            d = {}
            for i, name in enumerate(self.out_names):
                d[name] = np.asarray(outs[i]).reshape(
                    self.n_cores, *self.out_avals[i].shape)[c]
            out.append(d)
        return out


_CACHE = {}


def _get_runner():
    if "r" not in _CACHE:
        nc = build()
        _CACHE["r"] = SpmdRunner(nc, 8)
    return _CACHE["r"]


def kernel(**inputs):
    B, S, D, E = 4, 1024, 512, 8
    inputs = {k: np.asarray(v) for k, v in inputs.items()}
    r = _get_runner()
    in_maps = [make_in_map(inputs, e) for e in range(E)]
    r.set_inputs(in_maps)
    outs = r.run()
    res = r.results(outs)
    yT = res[0]["yT"].astype(np.float64)
    for e in range(1, E):
        yT += res[e]["yT"].astype(np.float64)
    return np.ascontiguousarray(yT.T).reshape(B, S, D).astype(np.float32)

